# revision 27
# baseline (speedup 1.0000x reference)
# MLA (Multi-head Latent Attention) Trainium2 kernel, 4-core SPMD.
#
# Measured reality of this axon-tunneled environment: piped per-iteration
# time ~= 0.16 ms fixed dispatch + ~1.2x the device makespan, SERIAL (the
# tunnel does not overlap submission with execution). So device work is
# the metric almost 1:1. This version shards over 4 cores: data-parallel
# over batch (B=2) x tensor-parallel over head halves (16 heads -> 2
# groups of 8), 8 heads per core in 2 passes of 4.
#
# Precision split (empirically validated, end-to-end ~6e-3 vs 2e-2 gate):
# everything that only feeds the SOFTMAX SCORES runs in fp8-e4m3 with
# DoubleRow matmuls (2x PE throughput): the q_c down-projection, the
# q/k/rope up-projections, and the score matmuls themselves (rope dims
# ride as a zero-padded second DoubleRow half, so scores cost ONE matmul).
# Softmax normalization cancels correlated pre-softmax quantization error.
# The value path (kv_c down-proj, V up-proj, P@V, output proj) stays bf16
# with f32 PSUM accumulation: fp8 there pushes past the gate.
#
# All inputs are packed into two blobs per core (bf16 + fp8; f32 consts
# ride in the bf16 blob as raw bytes and are bitcast on device), so each
# launch carries ~3 I/O buffers, not ~19.
#
# The V up-projection bias is folded into b_o on the host (after softmax,
# sum_k P = 1, so a per-d bias on V adds exactly b_vu @ W_o per row).
#
# Attention computes scores TRANSPOSED ([k, q]) so exp(scores) is directly
# the P^T operand PV needs; softmax denominators come from a ones-vector
# matmul on the PE and normalization happens on eviction. No max
# subtraction: |scores|*scale is bounded (~5) for any plausible input, so
# exp cannot overflow.
import numpy as np
from contextlib import ExitStack

B, S, HID = 2, 2048, 2048
NH, HD, RD = 16, 128, 64
KVC, QC = 512, 1536
NCORES = 4
HPC = 8                 # heads per core
HPP = 4                 # heads per pass
NPASS = 2
SCALE = 1.0 / float(np.sqrt(HD + RD))

# bf16 blob layout: (name, rows, cols, count): `count` slabs of [rows,cols]
_LAYOUT = [
    ("xT", 128, 2048, 16),
    ("w_down", 128, 2048, 4),     # kv slabs only (q runs fp8)
    ("w_vu", 128, 2048, 2),
    ("w_o", 128, 2048, 8),
    ("cos", 128, 2048, 1),
    ("sin", 128, 2048, 1),
    ("consts", 128, 360, 1),      # f32 [128,180] viewed as bf16 [128,360]
]
# fp8 blob layout (score path, DoubleRow pair layouts)
_LAYOUT8 = [
    ("wqd8", 128, 8 * 2 * 128, 12),  # per q slab: [128, 8pair, 2, 128]
    ("wku8", 128, 2 * 2 * 512, 2),   # per pass: [128, 2pair, 2, 4*128]
    ("wkr8", 128, 2 * 2 * 256, 2),   # per pass: [128, 2pair, 2, 2*128]
    ("wqu8", 128, 6 * 2 * 512, 2),   # per pass: [128, 6pair, 2, 4*128]
    ("wqr8", 128, 6 * 2 * 256, 2),   # per pass: [128, 6pair, 2, 2*128]
    ("perm8", 128, 128, 1),          # rope t1/t2 swap (p XOR 32) as matmul
]


def _offsets(layout):
    off, out = 0, {}
    for nm, r, c, n in layout:
        out[nm] = (off, r, c)
        off += r * c * n
    return out, off


_OFF, NBLOB = _offsets(_LAYOUT)
_OFF8, NBLOB8 = _offsets(_LAYOUT8)
# f32 column offsets inside consts: per-slab bias columns then causal
_BCOL = {"b_down": 0, "b_ku": 16, "b_kr": 24, "b_qu": 28, "b_qr": 36,
         "b_qd": 40}
_CAUSAL0, _NCONSTF = 52, 180

_CACHE = {}


def _build_nc():
    import concourse.bacc as bacc
    import concourse.mybir as mybir
    import concourse.tile as tile

    BF16 = mybir.dt.bfloat16
    F32 = mybir.dt.float32
    F8 = mybir.dt.float8e4
    AF = mybir.ActivationFunctionType
    DR = mybir.MatmulPerfMode.DoubleRow

    nc = bacc.Bacc("TRN2", target_bir_lowering=False, debug=False)

    blob = nc.dram_tensor("blob", [NBLOB], BF16, kind="ExternalInput")
    blob8 = nc.dram_tensor("blob8", [NBLOB8], F8, kind="ExternalInput")
    out_p = nc.dram_tensor("out_p", [S, HID], BF16, kind="ExternalOutput")

    def slab(nm, i=0):
        off, r, c = _OFF[nm]
        a = off + i * r * c
        return blob.ap()[a:a + r * c].rearrange("(p c) -> p c", p=r)

    def slab8(nm, i=0):
        off, r, c = _OFF8[nm]
        a = off + i * r * c
        return blob8.ap()[a:a + r * c].rearrange("(p c) -> p c", p=r)

    NB = S // 128        # 16 seq blocks
    with tile.TileContext(nc) as tc:
        with ExitStack() as sa:   # whole-kernel scope
            consts = sa.enter_context(tc.tile_pool(name="consts", bufs=1))
            ones_f = consts.tile([1, 128], F32, tag="onesf")
            nc.vector.memset(ones_f[:], 1.0)
            ones = consts.tile([1, 128], BF16, tag="ones")
            nc.vector.tensor_copy(ones[:], ones_f[:])
            onesc_f = consts.tile([128, 1], F32, tag="onescf")
            nc.vector.memset(onesc_f[:], 1.0)
            onesc = consts.tile([128, 1], BF16, tag="onesc")
            nc.vector.tensor_copy(onesc[:], onesc_f[:])
            consts_raw = consts.tile([128, 2 * _NCONSTF], BF16, tag="consts")

            def bias_ap(nm, o=0, n=1):
                c0 = _BCOL[nm] + o
                return consts_raw[:, 2 * c0:2 * (c0 + n)].bitcast(F32)

            causal_t = consts_raw[:, 2 * _CAUSAL0:2 * _NCONSTF].bitcast(F32)
            cos_t = consts.tile([128, S], BF16, tag="cos")
            sin_t = consts.tile([128, S], BF16, tag="sin")
            perm8_t = consts.tile([128, 128], F8, tag="perm8")

            def rope_chunk(raw, pair_e, pair_o, tmp_pool, pr, lo, hi,
                           ps_pool=None, dlo=None):
                # raw: fp8 [128, S] pair tile (rows: [h_even 64 | h_odd 64],
                # within head: [t1 32 | t2 32]); applies rope to columns
                # [lo:hi) and writes head-even rows 0:64 in place
                # (partition-aligned) into pair_e[0:64, 1, lo:hi], head-odd
                # rows 64:128 into pair_o via a byte DMA shift.
                # out = raw*cos + shuf(raw)*sin
                # The t1/t2 swap (partition p -> p XOR 32) runs as a fp8
                # permutation matmul on the PE when ps_pool is given (keeps
                # the latency-critical sync DMA queue free in B2); otherwise
                # as four partition-shifted byte DMAs.
                shuf = tmp_pool.tile([128, S], F8, tag=f"shuf{pr}",
                                     name=f"shuf{pr}")
                if ps_pool is not None:
                    pperm = ps_pool.tile([128, 512], F32, tag=f"perm{pr}",
                                         name=f"pperm{pr}")
                    for c in range(lo, hi, 512):
                        nc.tensor.matmul(pperm[:, 0:512], perm8_t[:],
                                         raw[:, c:c + 512],
                                         start=True, stop=True)
                        nc.scalar.copy(shuf[:, c:c + 512], pperm[:, 0:512])
                else:
                    for a in range(4):
                        src = (a ^ 1) * 32
                        nc.sync.dma_start(shuf[a * 32:(a + 1) * 32, lo:hi],
                                          raw[src:src + 32, lo:hi])
                t1 = tmp_pool.tile([128, S], BF16, tag=f"ropetmp{pr}",
                                   name=f"ropetmp{pr}")
                nc.vector.tensor_mul(t1[:, lo:hi], raw[:, lo:hi], cos_t[:, lo:hi])
                nc.vector.tensor_mul(shuf[:, lo:hi], shuf[:, lo:hi],
                                     sin_t[:, lo:hi])
                if dlo is None:
                    dlo = lo
                dhi = dlo + (hi - lo)
                nc.vector.tensor_add(pair_e[0:64, 1, dlo:dhi], t1[0:64, lo:hi],
                                     shuf[0:64, lo:hi])
                stage = tmp_pool.tile([128, S], F8, tag=f"ropest{pr}",
                                      name=f"ropest{pr}")
                nc.vector.tensor_add(stage[64:128, lo:hi], t1[64:128, lo:hi],
                                     shuf[64:128, lo:hi])
                nc.sync.dma_start(pair_o[0:64, 1, dlo:dhi], stage[64:128, lo:hi])

            # Latent projections stay in SBUF for the whole launch.
            # kvcT: bf16 (V path) + fp8 pair copy (score path); q_c: fp8 only.
            lat_pool = sa.enter_context(tc.tile_pool(name="lat", bufs=1))
            kvcT = [lat_pool.tile([128, S], BF16, tag=f"kvcT{i}", name=f"kvcT{i}")
                    for i in range(KVC // 128)]
            kvp8 = [lat_pool.tile([128, 2, S], F8, tag=f"kvp8_{i}",
                                  name=f"kvp8_{i}") for i in range(2)]
            qcp8 = [lat_pool.tile([128, 2, S], F8, tag=f"qcp8_{i}",
                                  name=f"qcp8_{i}") for i in range(6)]
            dram = sa.enter_context(tc.tile_pool(name="dram", bufs=1, space="DRAM"))
            ctx_d = dram.tile([HPP * 128, S], BF16)
            ctx1_pool = sa.enter_context(
                tc.tile_pool(name="ctx1", bufs=1, side="right"))
            ctx1 = [None] * HPP

            wps = sa.enter_context(tc.tile_pool(name="wps", bufs=1))
            # Phase-D prefetch targets (DMAs issued before C(1) so D never
            # stalls on its first weight/ctx tiles)
            dpre = sa.enter_context(tc.tile_pool(name="dpre", bufs=1,
                                                 side="right"))
            wo_t0 = dpre.tile([128, HID], BF16, tag="wo0", name="wo0")
            ctx_pre = [dpre.tile([128, HPP, 128], BF16, tag=f"cpre{st}",
                                 name=f"cpre{st}") for st in range(2)]

            def issue_pass_weights(p):
                # ordered by first use: B1 starts with k_r, B2 with q_r
                tiles = {}
                for nm, cols, f8 in (("wkr8", 2 * 2 * 256, True),
                                     ("wku8", 2 * 2 * 512, True),
                                     ("w_vu", 4 * 512, False),
                                     ("wqr8", 6 * 2 * 256, True),
                                     ("wqu8", 6 * 2 * 512, True)):
                    t = wps.tile([128, cols], F8 if f8 else BF16, tag=nm,
                                 name=f"{nm}{p}")
                    nc.sync.dma_start(t[:], slab8(nm, p) if f8 else slab(nm, p))
                    tiles[nm] = t
                return tiles

            # ---- Phase A: down projections. kv slabs (4) in bf16 with an
            # extra fp8 eviction; q slabs (12) fully fp8 via DoubleRow.
            with ExitStack() as s:
                xp = s.enter_context(tc.tile_pool(name="xp", bufs=16))
                x8p = s.enter_context(tc.tile_pool(name="x8p", bufs=1))
                wp = s.enter_context(tc.tile_pool(name="wA", bufs=3))
                ps = s.enter_context(tc.tile_pool(name="psA", bufs=2, space="PSUM"))

                # Critical-path first: kv weight slab 0, x tiles + consts,
                # remaining kv weight slabs, then the fp8 q-side inputs.
                wts = [None] * 4
                wts[0] = wp.tile([128, 16 * 128], BF16, tag="w", name="wA")
                nc.sync.dma_start(wts[0][:], slab("w_down", 0))
                xt = [xp.tile([128, S], BF16, tag="x", name="xt")]
                nc.sync.dma_start(xt[0][:], slab("xT", 0))
                nc.sync.dma_start(consts_raw[:], slab("consts"))
                for i in range(1, 16):
                    t = xp.tile([128, S], BF16, tag="x", name="xt")
                    nc.sync.dma_start(t[:], slab("xT", i))
                    xt.append(t)
                for ot in (1, 2, 3):
                    wts[ot] = wp.tile([128, 16 * 128], BF16, tag="w", name="wA")
                    nc.sync.dma_start(wts[ot][:], slab("w_down", ot))
                # x8 pair tiles are derived from the bf16 x tiles on the
                # DVE (idle during A): saves 4.2MB of input DMA per launch.
                x8t = []
                for j in range(8):
                    t = x8p.tile([128, 2, S], F8, tag=f"x8_{j}", name=f"x8_{j}")
                    nc.vector.tensor_copy(t[:, 0, :], xt[2 * j][:])
                    nc.vector.tensor_copy(t[:, 1, :], xt[2 * j + 1][:])
                    x8t.append(t)
                nc.gpsimd.dma_start(cos_t[:], slab("cos"))
                nc.gpsimd.dma_start(sin_t[:], slab("sin"))
                nc.gpsimd.dma_start(perm8_t[:], slab8("perm8"))
                passW = {0: issue_pass_weights(0)}
                # kv slabs: bf16, 16-cc accumulation, 4 s-chunks in parallel
                for ot in range(4):
                    pts = [ps.tile([128, 512], F32, tag=f"ps{sc}",
                                   name=f"psA{sc}") for sc in range(4)]
                    for hc in range(16):
                        for sc in range(4):
                            nc.tensor.matmul(
                                pts[sc][:], wts[ot][:, hc * 128:(hc + 1) * 128],
                                xt[hc][:, sc * 512:(sc + 1) * 512],
                                start=(hc == 0), stop=(hc == 15))
                    for sc in range(4):
                        nc.scalar.activation(
                            kvcT[ot][:, sc * 512:(sc + 1) * 512],
                            pts[sc][:], AF.Identity,
                            bias=bias_ap("b_down", ot))
                        nc.scalar.activation(
                            kvp8[ot // 2][:, ot % 2, sc * 512:(sc + 1) * 512],
                            pts[sc][:], AF.Identity,
                            bias=bias_ap("b_down", ot))
                # q slabs: fp8 DoubleRow (8 contraction pairs)
                w8p = s.enter_context(tc.tile_pool(name="w8A", bufs=2))
                for q in range(12):
                    wt8 = w8p.tile([128, 8, 2, 128], F8, tag="w8", name="w8A")
                    nc.sync.dma_start(wt8[:], slab8("wqd8", q).rearrange(
                        "p (j i m) -> p j i m", j=8, i=2))
                    pts = [ps.tile([128, 512], F32, tag=f"ps{sc}",
                                   name=f"psA{sc}") for sc in range(4)]
                    for j in range(8):
                        for sc in range(4):
                            nc.tensor.matmul(
                                pts[sc][:], wt8[:, j, :, :],
                                x8t[j][:, :, sc * 512:(sc + 1) * 512],
                                start=(j == 0), stop=(j == 7), perf_mode=DR)
                    for sc in range(4):
                        nc.scalar.activation(
                            qcp8[q // 2][:, q % 2, sc * 512:(sc + 1) * 512],
                            pts[sc][:], AF.Identity,
                            bias=bias_ap("b_qd", q))

            for p in range(NPASS):
              with ExitStack() as srep:  # pass scope: 4 heads
                if p not in passW:
                    passW[p] = issue_pass_weights(p)
                wku8_t, wvu_t, wkr8_t = (passW[p][k] for k in
                                         ("wku8", "w_vu", "wkr8"))
                wqu8_t, wqr8_t = (passW[p][k] for k in ("wqu8", "wqr8"))
                kv_out_pool = srep.enter_context(
                    tc.tile_pool(name="kv_out", bufs=1, side="right"))
                # Per-head score-operand pair tiles: [:, 0, :] = c-part,
                # [0:64, 1, :] = rope, [64:128, 1, :] = zeros.
                kpair = [kv_out_pool.tile([128, 2, S], F8, tag=f"kp{h}",
                                          name=f"kp{h}") for h in range(HPP)]
                V_all = kv_out_pool.tile([128, NB * HPP * HD], BF16, tag="V",
                                         name="V_all")
                q_out_pool = srep.enter_context(
                    tc.tile_pool(name="q_out", bufs=1, side="right"))
                qpair = [[q_out_pool.tile([128, 2, S // 2], F8,
                                          tag=f"qp{h}_{v}", name=f"qp{h}_{v}")
                          for v in range(2)] for h in range(HPP)]
                PT_p = srep.enter_context(tc.tile_pool(name="PTp", bufs=4))
                sm = srep.enter_context(tc.tile_pool(name="smC", bufs=4))
                if p == 0:
                    for h in range(HPP):
                        nc.vector.memset(kpair[h][64:128, 1, :], 0.0)
                        for v in range(2):
                            nc.vector.memset(qpair[h][v][64:128, 1, :], 0.0)

                # ---- Phase B1: kv-side up projections + k rope + V
                with ExitStack() as s:
                    tmp = s.enter_context(tc.tile_pool(name="tmpB1", bufs=1))
                    ps = s.enter_context(tc.tile_pool(name="psB1", bufs=2, space="PSUM"))
                    krraw = [tmp.tile([128, S], F8, tag=f"krraw{pr}",
                                      name=f"krraw{pr}") for pr in range(2)]
                    # k_r pairs first so their rope overlaps the k_c matmuls
                    for dst8, wsrc8, no, ow, bias, bo in (
                            (None, wkr8_t, 2, 256, "b_kr", 2 * p),
                            (kpair, wku8_t, HPP, 512, "b_ku", HPP * p)):
                        wt4 = wsrc8[:].rearrange("p (j i c) -> p j i c",
                                                 j=2, i=2)
                        for o in range(no):
                            pts = [ps.tile([128, 512], F32, tag=f"ps{sc}",
                                           name=f"psB{sc}") for sc in range(4)]
                            for pj in range(2):
                                for sc in range(4):
                                    nc.tensor.matmul(
                                        pts[sc][:],
                                        wt4[:, pj, :, o * 128:(o + 1) * 128],
                                        kvp8[pj][:, :, sc * 512:(sc + 1) * 512],
                                        start=(pj == 0), stop=(pj == 1),
                                        perf_mode=DR)
                            for sc in range(4):
                                d = (krraw[o][:, sc * 512:(sc + 1) * 512]
                                     if dst8 is None else
                                     dst8[o][:, 0, sc * 512:(sc + 1) * 512])
                                nc.scalar.activation(
                                    d, pts[sc][:], AF.Identity,
                                    bias=bias_ap(bias, bo + o))
                        if dst8 is None:
                            for pr in range(2):
                                rope_chunk(krraw[pr], kpair[2 * pr],
                                           kpair[2 * pr + 1], tmp, pr, 0, S)
                    for st in range(NB):      # V bf16 (bias folded into b_o)
                        pt = ps.tile([128, 512], F32, tag="ps0", name="psV")
                        for cc in range(4):
                            nc.tensor.matmul(
                                pt[:], kvcT[cc][:, st * 128:(st + 1) * 128],
                                wvu_t[:, cc * 512:(cc + 1) * 512],
                                start=(cc == 0), stop=(cc == 3))
                        nc.scalar.copy(V_all[:, st * 512:(st + 1) * 512], pt[:])

                # ---- Phase B2: q-side up projections, fp8 DoubleRow.
                with ExitStack() as s:
                    tmp2 = s.enter_context(tc.tile_pool(name="tmpB2", bufs=1))
                    ps = s.enter_context(tc.tile_pool(name="psB2", bufs=4, space="PSUM"))
                    ps_pm = s.enter_context(tc.tile_pool(name="psPm", bufs=1, space="PSUM"))
                    qrraw = [tmp2.tile([128, S], F8, tag=f"qrraw{pr}",
                                       name=f"qrraw{pr}") for pr in range(2)]
                    wqr4 = wqr8_t[:].rearrange("p (j i c) -> p j i c", j=6, i=2)
                    wqu4 = wqu8_t[:].rearrange("p (j i c) -> p j i c", j=6, i=2)
                    for sc in range(4):       # 512-wide s-chunks
                        for pr in range(2):
                            pt = ps.tile([128, 512], F32, tag="ps", name="psB2")
                            for pj in range(6):
                                nc.tensor.matmul(
                                    pt[:],
                                    wqr4[:, pj, :, pr * 128:(pr + 1) * 128],
                                    qcp8[pj][:, :, sc * 512:(sc + 1) * 512],
                                    start=(pj == 0), stop=(pj == 5),
                                    perf_mode=DR)
                            nc.scalar.activation(
                                qrraw[pr][:, sc * 512:(sc + 1) * 512], pt[:],
                                AF.Identity,
                                bias=bias_ap("b_qr", 2 * p + pr))
                        # rope this chunk now: overlaps the qu matmuls below
                        # instead of gating Phase C at the end of B2
                        for pr in range(2):
                            rope_chunk(qrraw[pr], qpair[2 * pr][sc // 2],
                                       qpair[2 * pr + 1][sc // 2], tmp2, pr,
                                       sc * 512, (sc + 1) * 512, ps_pool=ps_pm,
                                       dlo=(sc % 2) * 512)
                        for h in range(HPP):
                            pt = ps.tile([128, 512], F32, tag="ps", name="psB2")
                            for pj in range(6):
                                nc.tensor.matmul(
                                    pt[:],
                                    wqu4[:, pj, :, h * 128:(h + 1) * 128],
                                    qcp8[pj][:, :, sc * 512:(sc + 1) * 512],
                                    start=(pj == 0), stop=(pj == 5),
                                    perf_mode=DR)
                            nc.scalar.activation(
                                qpair[h][sc // 2][:, 0, (sc % 2) * 512:
                                                  (sc % 2) * 512 + 512], pt[:],
                                AF.Identity,
                                bias=bias_ap("b_qu", HPP * p + h))

                # Pre-issue next-pass weight DMAs (overlap Phase C) and,
                # before C(1), prefetch Phase D's first tiles (ctx_d pass-0
                # data is complete by end of C(0)).
                if p == 0:
                    passW[1] = issue_pass_weights(1)
                else:
                    nc.gpsimd.dma_start(wo_t0[:], slab("w_o", 0))
                    ctx_r_pre = ctx_d[:].rearrange("(h hp) s -> hp h s", hp=128)
                    for st in range(2):
                        nc.gpsimd.dma_start(
                            ctx_pre[st][:],
                            ctx_r_pre[:, :, st * 128:(st + 1) * 128])

                # ---- Phase C: causal attention, transposed-scores formulation.
                # scoresT[k, q] via ONE fp8 DoubleRow matmul per block (c-part
                # and rope-part are the two halves); PT = exp(scale * .) bf16;
                # ctxT[d, q] += V_j^T PT_j (bf16); den[1, q] += ones^T PT_j;
                # ctxT normalized by 1/den on eviction, parked in ctx_d (pass
                # 0) or SBUF (pass 1).
                with ExitStack() as s:
                    ps_sc = s.enter_context(tc.tile_pool(name="ps_sc", bufs=3, space="PSUM"))
                    ps_cx = s.enter_context(tc.tile_pool(name="ps_cx", bufs=2, space="PSUM"))
                    ps_dn = s.enter_context(tc.tile_pool(name="ps_dn", bufs=2, space="PSUM"))
                    ps_bc = s.enter_context(tc.tile_pool(name="ps_bc", bufs=1, space="PSUM"))
                    if p == 1:
                        for h in range(HPP):
                            ctx1[h] = ctx1_pool.tile(
                                [128, S], BF16, tag=f"ctx1_{h}", name=f"ctx1_{h}")
                    fin_prev = None
                    for g in range(4):
                        for h in range(HPP):
                            qlo = g * 512
                            pcx = ps_cx.tile([128, 512], F32, tag="ctx", name="pcx")
                            pden = ps_dn.tile([1, 512], F32, tag="den", name="pden")
                            njs = 4 * g + 4
                            # software-pipelined by one j: the PV/den matmuls
                            # for block j issue after block j+1's score
                            # matmuls, hiding the Exp latency from the PE.
                            pend = None

                            def flush(last):
                                jj, PTp_, c0p = pend
                                nc.tensor.matmul(
                                    pcx[:, c0p:512],
                                    V_all[:, jj * 512 + h * 128:
                                          jj * 512 + (h + 1) * 128],
                                    PTp_[:, c0p:512],
                                    start=(jj == 0), stop=last)
                                nc.tensor.matmul(
                                    pden[:, c0p:512], onesc[:], PTp_[:, c0p:512],
                                    start=(jj == 0), stop=last)

                            for j in range(njs):
                                c0 = max(0, j - 4 * g) * 128
                                pS = ps_sc.tile([128, 512], F32, tag="sT", name="pS")
                                ql = (g % 2) * 512
                                nc.tensor.matmul(
                                    pS[:, c0:512],
                                    kpair[h][:, :, j * 128:(j + 1) * 128],
                                    qpair[h][g // 2][:, :, ql + c0:ql + 512],
                                    start=True, stop=True, perf_mode=DR)
                                if j >= 4 * g:   # diagonal block
                                    nc.vector.tensor_add(
                                        pS[:, c0:c0 + 128], pS[:, c0:c0 + 128],
                                        causal_t)
                                PTt = PT_p.tile([128, 512], BF16, tag="PT", name="PTt")
                                nc.scalar.activation(
                                    PTt[:, c0:512], pS[:, c0:512], AF.Exp,
                                    scale=SCALE)
                                if pend is not None:
                                    flush(False)
                                pend = (j, PTt, c0)
                            flush(True)
                            rden = sm.tile([1, 512], BF16, tag="rden", name="rden")
                            with nc.allow_low_precision(
                                    reason="softmax rdenom as bf16 matmul operand"):
                                nc.vector.reciprocal(rden[:], pden[:])
                            # Finalization (rden broadcast + normalize) of the
                            # PREVIOUS group runs now: the pbc matmul never
                            # stalls the PE waiting on the DVE reciprocal.
                            if fin_prev is not None:
                                fin_prev()

                            def make_fin(pcx, rden, h, qlo):
                                def fin():
                                    pbc = ps_bc.tile([128, 512], F32, tag="bc",
                                                     name="pbc")
                                    nc.tensor.matmul(pbc[:], ones[:], rden[:],
                                                     start=True, stop=True)
                                    # DVE copy, NOT scalar.copy: the
                                    # Activation engine must stay on the Exp
                                    # table through all of C (each
                                    # Exp<->Identity switch costs ~1.3us).
                                    denb = sm.tile([128, 512], F32, tag="denb",
                                                   name="denb")
                                    nc.vector.tensor_copy(denb[:], pbc[:])
                                    if p == 1:
                                        nc.vector.tensor_mul(
                                            ctx1[h][:, qlo:qlo + 512],
                                            pcx[:], denb[:])
                                    else:
                                        cev = sm.tile([128, 512], BF16,
                                                      tag="cev", name="cev")
                                        nc.vector.tensor_mul(
                                            cev[:], pcx[:], denb[:])
                                        nc.sync.dma_start(
                                            ctx_d[h * 128:(h + 1) * 128,
                                                  qlo:qlo + 512], cev[:])
                                return fin

                            fin_prev = make_fin(pcx, rden, h, qlo)
                    fin_prev()

            # ---- Phase D: output projection (row-parallel partial, 8 heads).
            with ExitStack() as s:
                wop = s.enter_context(tc.tile_pool(name="wo", bufs=1))
                cxp = s.enter_context(tc.tile_pool(name="cxD", bufs=3))
                evd = s.enter_context(tc.tile_pool(name="evD", bufs=4))
                ps = s.enter_context(tc.tile_pool(name="psD", bufs=2, space="PSUM"))
                wo_t = [wo_t0] + [wop.tile([128, HID], BF16, tag=f"wo{h}",
                                           name=f"wo{h}")
                                  for h in range(1, HPC)]
                ctx_r = ctx_d[:].rearrange("(h hp) s -> hp h s", hp=128)
                ctx_tiles = ctx_pre
                for h in range(1, HPC):
                    nc.sync.dma_start(wo_t[h][:], slab("w_o", h))
                for st in range(NB):
                    if st < 2:
                        ctx_st = ctx_tiles[st]
                    else:
                        ctx_st = cxp.tile([128, HPP, 128], BF16, tag="cx",
                                          name="ctx_st")
                        nc.sync.dma_start(
                            ctx_st[:], ctx_r[:, :, st * 128:(st + 1) * 128])
                    pts = [ps.tile([128, 512], F32, tag=f"ps{oc}",
                                   name=f"psD{oc}") for oc in range(4)]
                    for h in range(HPC):
                        stat = (ctx_st[:, h, :] if h < HPP else
                                ctx1[h - HPP][:, st * 128:(st + 1) * 128])
                        for oc in range(4):
                            nc.tensor.matmul(
                                pts[oc][:], stat,
                                wo_t[h][:, oc * 512:(oc + 1) * 512],
                                start=(h == 0), stop=(h == HPC - 1))
                    for oc in range(4):
                        ev = evd.tile([128, 512], BF16, tag="evD", name="evD")
                        nc.scalar.copy(ev[:], pts[oc][:])
                        nc.sync.dma_start(
                            out_p.ap()[st * 128:(st + 1) * 128,
                                       oc * 512:(oc + 1) * 512], ev[:])

    nc.compile()
    return nc


def _host_inputs(inputs):
    import ml_dtypes
    f32 = np.float32
    bf16 = ml_dtypes.bfloat16
    fp8 = ml_dtypes.float8_e4m3

    def b16(a):
        return np.ascontiguousarray(np.asarray(a, f32).astype(bf16))

    def b8(a):
        return np.ascontiguousarray(np.asarray(a, f32).astype(fp8))

    x = np.asarray(inputs["x"], dtype=f32)
    W_kvd, b_kvd = inputs["W_kvd"], np.asarray(inputs["b_kvd"], f32)
    W_ku, b_ku = inputs["W_ku"], np.asarray(inputs["b_ku"], f32)
    W_vu, b_vu = inputs["W_vu"], np.asarray(inputs["b_vu"], f32)
    W_kr, b_kr = inputs["W_kr"], np.asarray(inputs["b_kr"], f32)
    W_qd, b_qd = inputs["W_qd"], np.asarray(inputs["b_qd"], f32)
    W_qu, b_qu = inputs["W_qu"], np.asarray(inputs["b_qu"], f32)
    W_qr, b_qr = inputs["W_qr"], np.asarray(inputs["b_qr"], f32)
    W_o = inputs["W_o"]

    xT = [np.ascontiguousarray(np.asarray(x[b]).T) for b in range(B)]

    inv_freq = (1.0 / (10000.0 ** (np.arange(0, RD, 2, dtype=np.float64) / RD)))
    ang = np.arange(S, dtype=np.float64)[:, None] * inv_freq[None, :]  # [S, 32]
    cosT = np.cos(ang).T.astype(f32)   # [32, S]
    sinT = np.sin(ang).T.astype(f32)
    cospair = b16(np.tile(cosT, (4, 1)))                               # [128, S]
    sinpair = b16(np.concatenate([-sinT, sinT, -sinT, sinT], axis=0))  # [128, S]
    # transposed-scores causal mask: mask k > q within the diagonal block
    causal = np.where(np.tril(np.ones((128, 128), bool), -1),
                      f32(-1e9), f32(0.0)).astype(f32)

    def tile_pack(W, n_ot):
        # [K, n_ot*cols] -> [n_ot, 128, (K/128)*cols] cc-major stationary
        W = np.asarray(W, f32)
        K, C = W.shape
        ncc = K // 128
        cols = C // n_ot
        return (W.reshape(ncc, 128, n_ot, cols).transpose(2, 1, 0, 3)
                .reshape(n_ot, 128, ncc * cols))

    def pack8(W, n_ot):
        # fp8 pair-stationary: [n_ot, 128, ncc*cols] -> [n_ot, 128, ncc/2,
        # 2, cols] flattened (cc pairs interleaved per DoubleRow half)
        t = tile_pack(W, n_ot)
        n, p, cc = t.shape
        return b8(t).reshape(n, p, -1)   # layout already cc-major pairs

    kvdT = b16(tile_pack(W_kvd, 4))       # [4, 128, 2048]
    qdT8 = pack8(W_qd, 12)                # [12, 128, 2048] (8 pairs x 2 x 128)
    b_down_c = b_kvd.reshape(4, 128).T    # [128, 4] (kv slabs only)
    b_qd_c = b_qd.reshape(12, 128).T      # [128, 12]

    in_maps = []
    for c in range(NCORES):
        b, g = c // 2, c % 2
        hc = slice(HPC * g * HD, (HPC * g + HPC) * HD)    # head cols (128 each)
        rc = slice(HPC * g * RD, (HPC * g + HPC) * RD)    # rope cols (64 each)
        consts_f = np.concatenate([
            np.pad(b_down_c, ((0, 0), (0, 12))),          # b_down: 16 cols
            np.ascontiguousarray(b_ku[hc].reshape(HPC, 128).T),
            np.ascontiguousarray(b_kr[rc].reshape(HPC // 2, 128).T),
            np.ascontiguousarray(b_qu[hc].reshape(HPC, 128).T),
            np.ascontiguousarray(b_qr[rc].reshape(HPC // 2, 128).T),
            b_qd_c,                                        # 12 cols
            causal,
        ], axis=1).astype(f32)                             # [128, 184]
        assert consts_f.shape == (128, _NCONSTF), consts_f.shape
        parts = {
            "xT": b16(xT[b]),
            "w_down": kvdT,
            "w_vu": b16(tile_pack(np.asarray(W_vu, f32)[:, hc], NPASS)),
            "w_o": b16(np.asarray(W_o, f32)[hc, :]),
            "cos": cospair,
            "sin": sinpair,
            "consts": np.ascontiguousarray(consts_f).view(bf16),
        }
        perm = np.zeros((128, 128), np.float32)
        for pp in range(128):
            perm[pp ^ 32, pp] = 1.0
        parts8 = {
            "perm8": b8(perm),
            "wqd8": qdT8,
            "wku8": pack8(np.asarray(W_ku, f32)[:, hc], NPASS),
            "wkr8": pack8(np.asarray(W_kr, f32)[:, rc], NPASS),
            "wqu8": pack8(np.asarray(W_qu, f32)[:, hc], NPASS),
            "wqr8": pack8(np.asarray(W_qr, f32)[:, rc], NPASS),
        }
        import ml_dtypes as _md
        blob = np.empty(NBLOB, bf16)
        off = 0
        for nm, r, cc_, n in _LAYOUT:
            a = np.ascontiguousarray(parts[nm]).reshape(-1)
            assert a.size == r * cc_ * n, (nm, a.size, r * cc_ * n)
            blob[off:off + a.size] = a
            off += a.size
        blob8 = np.empty(NBLOB8, fp8)
        off = 0
        for nm, r, cc_, n in _LAYOUT8:
            a = np.ascontiguousarray(parts8[nm]).reshape(-1)
            assert a.size == r * cc_ * n, (nm, a.size, r * cc_ * n)
            blob8[off:off + a.size] = a
            off += a.size
        in_maps.append({"blob": blob, "blob8": blob8})
    # V-bias fold: sum_k P = 1 after softmax, so +b_vu on V adds exactly
    # b_vu @ W_o to every output row; host adds it with b_o.
    b_eff = (np.asarray(inputs["b_o"], f32)
             + b_vu.astype(f32) @ np.asarray(W_o, f32))
    return in_maps, b_eff


def _run(inputs, trace=False):
    from concourse import bass_utils
    if "nc" not in _CACHE:
        _CACHE["nc"] = _build_nc()
    nc = _CACHE["nc"]
    in_maps, b_eff = _host_inputs(inputs)
    res = bass_utils.run_bass_kernel_spmd(
        nc, in_maps, core_ids=list(range(NCORES)), trace=trace)
    out = np.zeros((B, S, HID), np.float32)
    for c in range(NCORES):
        out[c // 2] += np.asarray(res.results[c]["out_p"], np.float32)
    out += b_eff[None, None, :]
    return out, res


def kernel(**inputs) -> np.ndarray:
    out, _ = _run(inputs, trace=False)
    return out


def bench(inputs, iters=10):
    """Time NEFF execution on the cores via PJRT, excluding host->device
    transfers and compile. Returns (best_ns, info)."""
    import time
    import jax
    from jax.experimental.shard_map import shard_map
    from jax.sharding import Mesh, PartitionSpec
    import concourse.mybir as mybir
    from concourse.bass2jax import (_bass_exec_p, install_neuronx_cc_hook,
                                    partition_id_tensor)

    if "nc" not in _CACHE:
        _CACHE["nc"] = _build_nc()
    nc = _CACHE["nc"]
    in_maps, _ = _host_inputs(inputs)
    install_neuronx_cc_hook()

    partition_name = nc.partition_id_tensor.name if nc.partition_id_tensor else None
    in_names, out_names, out_avals, zero_outs = [], [], [], []
    for alloc in nc.m.functions[0].allocations:
        if not isinstance(alloc, mybir.MemoryLocationSet):
            continue
        name = alloc.memorylocations[0].name
        if alloc.kind == "ExternalInput":
            if name != partition_name:
                in_names.append(name)
        elif alloc.kind == "ExternalOutput":
            out_names.append(name)
            shape = tuple(alloc.tensor_shape)
            dtype = mybir.dt.np(alloc.dtype)
            out_avals.append(jax.core.ShapedArray(shape, dtype))
            zero_outs.append(np.zeros(shape, dtype))
    n_params = len(in_names)
    all_names = list(in_names) + list(out_names)
    if partition_name is not None:
        all_names.append(partition_name)

    def _body(*args):
        operands = list(args)
        if partition_name is not None:
            operands.append(partition_id_tensor())
        outs = _bass_exec_p.bind(
            *operands,
            out_avals=tuple(out_avals),
            in_names=tuple(all_names),
            out_names=tuple(out_names),
            lowering_input_output_aliases=(),
            sim_require_finite=True,
            sim_require_nnan=True,
            nc=nc,
        )
        return tuple(outs)

    n = NCORES
    devices = jax.devices()[:n]
    mesh = Mesh(np.asarray(devices), ("core",))
    nin = n_params + len(out_names)
    fn = jax.jit(shard_map(
        _body, mesh=mesh,
        in_specs=(PartitionSpec("core"),) * nin,
        out_specs=(PartitionSpec("core"),) * len(out_names),
        check_rep=False), keep_unused=True)
    concat_in = [np.concatenate([np.asarray(in_maps[c][k]) for c in range(n)], 0)
                 for k in in_names]
    concat_zeros = [np.zeros((n * z.shape[0], *z.shape[1:]), z.dtype)
                    for z in zero_outs]
    sharding = jax.sharding.NamedSharding(mesh, PartitionSpec("core"))
    dev_in = [jax.device_put(a, sharding) for a in concat_in + concat_zeros]
    out = fn(*dev_in)  # warm-up/compile
    jax.block_until_ready(out)
    times = []
    for _ in range(iters):
        t0 = time.perf_counter()
        out = fn(*dev_in)
        jax.block_until_ready(out)
        times.append((time.perf_counter() - t0) * 1e9)

    def run_k(k):
        t0 = time.perf_counter()
        outs = [fn(*dev_in) for _ in range(k)]
        jax.block_until_ready(outs)
        return (time.perf_counter() - t0) * 1e9

    # pipelined: K async submissions, block once; amortizes tunnel latency.
    K1, K2 = 8, 48
    piped_samples, tKs = [], []
    for _ in range(7):
        a = run_k(K1)
        b = run_k(K2)
        tKs.append((a, b))
        piped_samples.append((b - a) / (K2 - K1))
    piped = sorted(piped_samples)[len(piped_samples) // 2]
    sustained = min(b / K2 for _, b in tKs)
    t1 = min(times)
    best = min(times + [sustained])
    if 0 < piped < sustained:
        best = min(best, piped)
    return best, {"serial": times, "tK": tKs[-1][1], "t1": t1,
                  "piped": piped, "piped_samples": piped_samples,
                  "sustained": sustained}


# revision 28
# speedup vs baseline: 1.0106x; 1.0106x over previous
# MLA (Multi-head Latent Attention) Trainium2 kernel, 4-core SPMD.
#
# Measured reality of this axon-tunneled environment: piped per-iteration
# time ~= 0.16 ms fixed dispatch + ~1.2x the device makespan, SERIAL (the
# tunnel does not overlap submission with execution). So device work is
# the metric almost 1:1. This version shards over 4 cores: data-parallel
# over batch (B=2) x tensor-parallel over head halves (16 heads -> 2
# groups of 8), 8 heads per core in 2 passes of 4.
#
# Precision split (empirically validated, end-to-end ~6e-3 vs 2e-2 gate):
# everything that only feeds the SOFTMAX SCORES runs in fp8-e4m3 with
# DoubleRow matmuls (2x PE throughput): the q_c down-projection, the
# q/k/rope up-projections, and the score matmuls themselves (rope dims
# ride as a zero-padded second DoubleRow half, so scores cost ONE matmul).
# Softmax normalization cancels correlated pre-softmax quantization error.
# The value path (kv_c down-proj, V up-proj, P@V, output proj) stays bf16
# with f32 PSUM accumulation: fp8 there pushes past the gate.
#
# All inputs are packed into two blobs per core (bf16 + fp8; f32 consts
# ride in the bf16 blob as raw bytes and are bitcast on device), so each
# launch carries ~3 I/O buffers, not ~19.
#
# The V up-projection bias is folded into b_o on the host (after softmax,
# sum_k P = 1, so a per-d bias on V adds exactly b_vu @ W_o per row).
#
# Attention computes scores TRANSPOSED ([k, q]) so exp(scores) is directly
# the P^T operand PV needs; softmax denominators come from a ones-vector
# matmul on the PE and normalization happens on eviction. No max
# subtraction: |scores|*scale is bounded (~5) for any plausible input, so
# exp cannot overflow.
import numpy as np
from contextlib import ExitStack

B, S, HID = 2, 2048, 2048
NH, HD, RD = 16, 128, 64
KVC, QC = 512, 1536
NCORES = 4
HPC = 8                 # heads per core
HPP = 4                 # heads per pass
NPASS = 2
SCALE = 1.0 / float(np.sqrt(HD + RD))

# bf16 blob layout: (name, rows, cols, count): `count` slabs of [rows,cols]
_LAYOUT = [
    ("xT", 128, 2048, 16),
    ("w_down", 128, 2048, 4),     # kv slabs only (q runs fp8)
    ("w_vu", 128, 2048, 2),
    ("w_o", 128, 2048, 8),
    ("cos", 128, 2048, 1),
    ("sin", 128, 2048, 1),
    ("consts", 128, 360, 1),      # f32 [128,180] viewed as bf16 [128,360]
]
# fp8 blob layout (score path, DoubleRow pair layouts)
_LAYOUT8 = [
    ("wqd8", 128, 8 * 2 * 128, 12),  # per q slab: [128, 8pair, 2, 128]
    ("wku8", 128, 2 * 2 * 512, 2),   # per pass: [128, 2pair, 2, 4*128]
    ("wkr8", 128, 2 * 2 * 256, 2),   # per pass: [128, 2pair, 2, 2*128]
    ("wqu8", 128, 6 * 2 * 512, 2),   # per pass: [128, 6pair, 2, 4*128]
    ("wqr8", 128, 6 * 2 * 256, 2),   # per pass: [128, 6pair, 2, 2*128]
    ("perm8", 128, 128, 1),          # rope t1/t2 swap (p XOR 32) as matmul
]


def _offsets(layout):
    off, out = 0, {}
    for nm, r, c, n in layout:
        out[nm] = (off, r, c)
        off += r * c * n
    return out, off


_OFF, NBLOB = _offsets(_LAYOUT)
_OFF8, NBLOB8 = _offsets(_LAYOUT8)
# f32 column offsets inside consts: per-slab bias columns then causal
_BCOL = {"b_down": 0, "b_ku": 16, "b_kr": 24, "b_qu": 28, "b_qr": 36,
         "b_qd": 40}
_CAUSAL0, _NCONSTF = 52, 180

_CACHE = {}


def _build_nc():
    import concourse.bacc as bacc
    import concourse.mybir as mybir
    import concourse.tile as tile

    BF16 = mybir.dt.bfloat16
    F32 = mybir.dt.float32
    F8 = mybir.dt.float8e4
    AF = mybir.ActivationFunctionType
    DR = mybir.MatmulPerfMode.DoubleRow

    nc = bacc.Bacc("TRN2", target_bir_lowering=False, debug=False)

    blob = nc.dram_tensor("blob", [NBLOB], BF16, kind="ExternalInput")
    blob8 = nc.dram_tensor("blob8", [NBLOB8], F8, kind="ExternalInput")
    out_p = nc.dram_tensor("out_p", [S, HID], BF16, kind="ExternalOutput")

    def slab(nm, i=0):
        off, r, c = _OFF[nm]
        a = off + i * r * c
        return blob.ap()[a:a + r * c].rearrange("(p c) -> p c", p=r)

    def slab8(nm, i=0):
        off, r, c = _OFF8[nm]
        a = off + i * r * c
        return blob8.ap()[a:a + r * c].rearrange("(p c) -> p c", p=r)

    NB = S // 128        # 16 seq blocks
    with tile.TileContext(nc) as tc:
        with ExitStack() as sa:   # whole-kernel scope
            consts = sa.enter_context(tc.tile_pool(name="consts", bufs=1))
            ones_f = consts.tile([1, 128], F32, tag="onesf")
            nc.vector.memset(ones_f[:], 1.0)
            ones = consts.tile([1, 128], BF16, tag="ones")
            nc.vector.tensor_copy(ones[:], ones_f[:])
            onesc_f = consts.tile([128, 1], F32, tag="onescf")
            nc.vector.memset(onesc_f[:], 1.0)
            onesc = consts.tile([128, 1], BF16, tag="onesc")
            nc.vector.tensor_copy(onesc[:], onesc_f[:])
            consts_raw = consts.tile([128, 2 * _NCONSTF], BF16, tag="consts")

            def bias_ap(nm, o=0, n=1):
                c0 = _BCOL[nm] + o
                return consts_raw[:, 2 * c0:2 * (c0 + n)].bitcast(F32)

            causal_t = consts_raw[:, 2 * _CAUSAL0:2 * _NCONSTF].bitcast(F32)
            cos_t = consts.tile([128, S], BF16, tag="cos")
            sin_t = consts.tile([128, S], BF16, tag="sin")
            perm8_t = consts.tile([128, 128], F8, tag="perm8")

            def rope_chunk(raw, pair_e, pair_o, tmp_pool, pr, lo, hi,
                           ps_pool=None):
                # raw: fp8 [128, S] pair tile (rows: [h_even 64 | h_odd 64],
                # within head: [t1 32 | t2 32]); applies rope to columns
                # [lo:hi) and writes head-even rows 0:64 in place
                # (partition-aligned) into pair_e[0:64, 1, lo:hi], head-odd
                # rows 64:128 into pair_o via a byte DMA shift.
                # out = raw*cos + shuf(raw)*sin
                # The t1/t2 swap (partition p -> p XOR 32) runs as a fp8
                # permutation matmul on the PE when ps_pool is given (keeps
                # the latency-critical sync DMA queue free in B2); otherwise
                # as four partition-shifted byte DMAs.
                shuf = tmp_pool.tile([128, S], F8, tag=f"shuf{pr}",
                                     name=f"shuf{pr}")
                if ps_pool is not None:
                    pperm = ps_pool.tile([128, 512], F32, tag=f"perm{pr}",
                                         name=f"pperm{pr}")
                    for c in range(lo, hi, 512):
                        nc.tensor.matmul(pperm[:, 0:512], perm8_t[:],
                                         raw[:, c:c + 512],
                                         start=True, stop=True)
                        nc.scalar.copy(shuf[:, c:c + 512], pperm[:, 0:512])
                else:
                    for a in range(4):
                        src = (a ^ 1) * 32
                        nc.sync.dma_start(shuf[a * 32:(a + 1) * 32, lo:hi],
                                          raw[src:src + 32, lo:hi])
                t1 = tmp_pool.tile([128, S], BF16, tag=f"ropetmp{pr}",
                                   name=f"ropetmp{pr}")
                nc.vector.tensor_mul(t1[:, lo:hi], raw[:, lo:hi], cos_t[:, lo:hi])
                nc.vector.tensor_mul(shuf[:, lo:hi], shuf[:, lo:hi],
                                     sin_t[:, lo:hi])
                nc.vector.tensor_add(pair_e[0:64, 1, lo:hi], t1[0:64, lo:hi],
                                     shuf[0:64, lo:hi])
                stage = tmp_pool.tile([128, S], F8, tag=f"ropest{pr}",
                                      name=f"ropest{pr}")
                nc.vector.tensor_add(stage[64:128, lo:hi], t1[64:128, lo:hi],
                                     shuf[64:128, lo:hi])
                nc.sync.dma_start(pair_o[0:64, 1, lo:hi], stage[64:128, lo:hi])

            # Latent projections stay in SBUF for the whole launch.
            # kvcT: bf16 (V path) + fp8 pair copy (score path); q_c: fp8 only.
            lat_pool = sa.enter_context(tc.tile_pool(name="lat", bufs=1))
            kvcT = [lat_pool.tile([128, S], BF16, tag=f"kvcT{i}", name=f"kvcT{i}")
                    for i in range(KVC // 128)]
            kvp8 = [lat_pool.tile([128, 2, S], F8, tag=f"kvp8_{i}",
                                  name=f"kvp8_{i}") for i in range(2)]
            qcp8 = [lat_pool.tile([128, 2, S], F8, tag=f"qcp8_{i}",
                                  name=f"qcp8_{i}") for i in range(6)]
            dram = sa.enter_context(tc.tile_pool(name="dram", bufs=1, space="DRAM"))
            ctx_d = dram.tile([HPP * 128, S], BF16)
            ctx1_pool = sa.enter_context(
                tc.tile_pool(name="ctx1", bufs=1, side="right"))
            ctx1 = [None] * HPP

            wps = sa.enter_context(tc.tile_pool(name="wps", bufs=1))
            # Phase-D prefetch targets (DMAs issued before C(1) so D never
            # stalls on its first weight/ctx tiles)
            dpre = sa.enter_context(tc.tile_pool(name="dpre", bufs=1,
                                                 side="right"))
            wo_t0 = dpre.tile([128, HID], BF16, tag="wo0", name="wo0")
            ctx_pre = [dpre.tile([128, HPP, 128], BF16, tag=f"cpre{st}",
                                 name=f"cpre{st}") for st in range(2)]

            def issue_pass_weights(p):
                # ordered by first use: B1 starts with k_r, B2 with q_r
                tiles = {}
                for nm, cols, f8 in (("wkr8", 2 * 2 * 256, True),
                                     ("wku8", 2 * 2 * 512, True),
                                     ("w_vu", 4 * 512, False),
                                     ("wqr8", 6 * 2 * 256, True),
                                     ("wqu8", 6 * 2 * 512, True)):
                    t = wps.tile([128, cols], F8 if f8 else BF16, tag=nm,
                                 name=f"{nm}{p}")
                    nc.sync.dma_start(t[:], slab8(nm, p) if f8 else slab(nm, p))
                    tiles[nm] = t
                return tiles

            # ---- Phase A: down projections. kv slabs (4) in bf16 with an
            # extra fp8 eviction; q slabs (12) fully fp8 via DoubleRow.
            with ExitStack() as s:
                xp = s.enter_context(tc.tile_pool(name="xp", bufs=16))
                x8p = s.enter_context(tc.tile_pool(name="x8p", bufs=1))
                wp = s.enter_context(tc.tile_pool(name="wA", bufs=3))
                ps = s.enter_context(tc.tile_pool(name="psA", bufs=2, space="PSUM"))

                # Critical-path first: kv weight slab 0, x tiles + consts,
                # remaining kv weight slabs, then the fp8 q-side inputs.
                wts = [None] * 4
                wts[0] = wp.tile([128, 16 * 128], BF16, tag="w", name="wA")
                nc.sync.dma_start(wts[0][:], slab("w_down", 0))
                xt = [xp.tile([128, S], BF16, tag="x", name="xt")]
                nc.sync.dma_start(xt[0][:], slab("xT", 0))
                nc.sync.dma_start(consts_raw[:], slab("consts"))
                for i in range(1, 16):
                    t = xp.tile([128, S], BF16, tag="x", name="xt")
                    nc.sync.dma_start(t[:], slab("xT", i))
                    xt.append(t)
                for ot in (1, 2, 3):
                    wts[ot] = wp.tile([128, 16 * 128], BF16, tag="w", name="wA")
                    nc.sync.dma_start(wts[ot][:], slab("w_down", ot))
                # x8 pair tiles are derived from the bf16 x tiles on the
                # DVE (idle during A): saves 4.2MB of input DMA per launch.
                x8t = []
                for j in range(8):
                    t = x8p.tile([128, 2, S], F8, tag=f"x8_{j}", name=f"x8_{j}")
                    nc.vector.tensor_copy(t[:, 0, :], xt[2 * j][:])
                    nc.vector.tensor_copy(t[:, 1, :], xt[2 * j + 1][:])
                    x8t.append(t)
                nc.gpsimd.dma_start(cos_t[:], slab("cos"))
                nc.gpsimd.dma_start(sin_t[:], slab("sin"))
                nc.gpsimd.dma_start(perm8_t[:], slab8("perm8"))
                passW = {0: issue_pass_weights(0)}
                # kv slabs: bf16, 16-cc accumulation, 4 s-chunks in parallel
                for ot in range(4):
                    pts = [ps.tile([128, 512], F32, tag=f"ps{sc}",
                                   name=f"psA{sc}") for sc in range(4)]
                    for hc in range(16):
                        for sc in range(4):
                            nc.tensor.matmul(
                                pts[sc][:], wts[ot][:, hc * 128:(hc + 1) * 128],
                                xt[hc][:, sc * 512:(sc + 1) * 512],
                                start=(hc == 0), stop=(hc == 15))
                    for sc in range(4):
                        nc.scalar.activation(
                            kvcT[ot][:, sc * 512:(sc + 1) * 512],
                            pts[sc][:], AF.Identity,
                            bias=bias_ap("b_down", ot))
                        nc.scalar.activation(
                            kvp8[ot // 2][:, ot % 2, sc * 512:(sc + 1) * 512],
                            pts[sc][:], AF.Identity,
                            bias=bias_ap("b_down", ot))
                # q slabs: fp8 DoubleRow (8 contraction pairs)
                w8p = s.enter_context(tc.tile_pool(name="w8A", bufs=2))
                for q in range(12):
                    wt8 = w8p.tile([128, 8, 2, 128], F8, tag="w8", name="w8A")
                    nc.sync.dma_start(wt8[:], slab8("wqd8", q).rearrange(
                        "p (j i m) -> p j i m", j=8, i=2))
                    pts = [ps.tile([128, 512], F32, tag=f"ps{sc}",
                                   name=f"psA{sc}") for sc in range(4)]
                    for j in range(8):
                        for sc in range(4):
                            nc.tensor.matmul(
                                pts[sc][:], wt8[:, j, :, :],
                                x8t[j][:, :, sc * 512:(sc + 1) * 512],
                                start=(j == 0), stop=(j == 7), perf_mode=DR)
                    for sc in range(4):
                        nc.scalar.activation(
                            qcp8[q // 2][:, q % 2, sc * 512:(sc + 1) * 512],
                            pts[sc][:], AF.Identity,
                            bias=bias_ap("b_qd", q))

            for p in range(NPASS):
              with ExitStack() as srep:  # pass scope: 4 heads
                if p not in passW:
                    passW[p] = issue_pass_weights(p)
                wku8_t, wvu_t, wkr8_t = (passW[p][k] for k in
                                         ("wku8", "w_vu", "wkr8"))
                wqu8_t, wqr8_t = (passW[p][k] for k in ("wqu8", "wqr8"))
                kv_out_pool = srep.enter_context(
                    tc.tile_pool(name="kv_out", bufs=1, side="right"))
                # Per-head score-operand pair tiles: [:, 0, :] = c-part,
                # [0:64, 1, :] = rope, [64:128, 1, :] = zeros.
                kpair = [kv_out_pool.tile([128, 2, S], F8, tag=f"kp{h}",
                                          name=f"kp{h}") for h in range(HPP)]
                V_all = kv_out_pool.tile([128, NB * HPP * HD], BF16, tag="V",
                                         name="V_all")
                q_out_pool = srep.enter_context(
                    tc.tile_pool(name="q_out", bufs=1, side="right"))
                qpair = [q_out_pool.tile([128, 2, S], F8, tag=f"qp{h}",
                                         name=f"qp{h}") for h in range(HPP)]
                if p == 0:
                    for h in range(HPP):
                        nc.vector.memset(kpair[h][64:128, 1, :], 0.0)
                        nc.vector.memset(qpair[h][64:128, 1, :], 0.0)

                # ---- Phase B1: kv-side up projections + k rope + V
                with ExitStack() as s:
                    tmp = s.enter_context(tc.tile_pool(name="tmpB1", bufs=1))
                    ps = s.enter_context(tc.tile_pool(name="psB1", bufs=2, space="PSUM"))
                    krraw = [tmp.tile([128, S], F8, tag=f"krraw{pr}",
                                      name=f"krraw{pr}") for pr in range(2)]
                    # k_r pairs first so their rope overlaps the k_c matmuls
                    for dst8, wsrc8, no, ow, bias, bo in (
                            (None, wkr8_t, 2, 256, "b_kr", 2 * p),
                            (kpair, wku8_t, HPP, 512, "b_ku", HPP * p)):
                        wt4 = wsrc8[:].rearrange("p (j i c) -> p j i c",
                                                 j=2, i=2)
                        for o in range(no):
                            pts = [ps.tile([128, 512], F32, tag=f"ps{sc}",
                                           name=f"psB{sc}") for sc in range(4)]
                            for pj in range(2):
                                for sc in range(4):
                                    nc.tensor.matmul(
                                        pts[sc][:],
                                        wt4[:, pj, :, o * 128:(o + 1) * 128],
                                        kvp8[pj][:, :, sc * 512:(sc + 1) * 512],
                                        start=(pj == 0), stop=(pj == 1),
                                        perf_mode=DR)
                            for sc in range(4):
                                d = (krraw[o][:, sc * 512:(sc + 1) * 512]
                                     if dst8 is None else
                                     dst8[o][:, 0, sc * 512:(sc + 1) * 512])
                                nc.scalar.activation(
                                    d, pts[sc][:], AF.Identity,
                                    bias=bias_ap(bias, bo + o))
                        if dst8 is None:
                            for pr in range(2):
                                rope_chunk(krraw[pr], kpair[2 * pr],
                                           kpair[2 * pr + 1], tmp, pr, 0, S)
                    for st in range(NB):      # V bf16 (bias folded into b_o)
                        pt = ps.tile([128, 512], F32, tag="ps0", name="psV")
                        for cc in range(4):
                            nc.tensor.matmul(
                                pt[:], kvcT[cc][:, st * 128:(st + 1) * 128],
                                wvu_t[:, cc * 512:(cc + 1) * 512],
                                start=(cc == 0), stop=(cc == 3))
                        nc.scalar.copy(V_all[:, st * 512:(st + 1) * 512], pt[:])

                # ---- Phase B2: q-side up projections, fp8 DoubleRow.
                with ExitStack() as s:
                    tmp2 = s.enter_context(tc.tile_pool(name="tmpB2", bufs=1))
                    ps = s.enter_context(tc.tile_pool(name="psB2", bufs=4, space="PSUM"))
                    ps_pm = s.enter_context(tc.tile_pool(name="psPm", bufs=1, space="PSUM"))
                    qrraw = [tmp2.tile([128, S], F8, tag=f"qrraw{pr}",
                                       name=f"qrraw{pr}") for pr in range(2)]
                    wqr4 = wqr8_t[:].rearrange("p (j i c) -> p j i c", j=6, i=2)
                    wqu4 = wqu8_t[:].rearrange("p (j i c) -> p j i c", j=6, i=2)
                    for sc in range(4):       # 512-wide s-chunks
                        for pr in range(2):
                            pt = ps.tile([128, 512], F32, tag="ps", name="psB2")
                            for pj in range(6):
                                nc.tensor.matmul(
                                    pt[:],
                                    wqr4[:, pj, :, pr * 128:(pr + 1) * 128],
                                    qcp8[pj][:, :, sc * 512:(sc + 1) * 512],
                                    start=(pj == 0), stop=(pj == 5),
                                    perf_mode=DR)
                            nc.scalar.activation(
                                qrraw[pr][:, sc * 512:(sc + 1) * 512], pt[:],
                                AF.Identity,
                                bias=bias_ap("b_qr", 2 * p + pr))
                        # rope this chunk now: overlaps the qu matmuls below
                        # instead of gating Phase C at the end of B2
                        for pr in range(2):
                            rope_chunk(qrraw[pr], qpair[2 * pr],
                                       qpair[2 * pr + 1], tmp2, pr,
                                       sc * 512, (sc + 1) * 512, ps_pool=ps_pm)
                        for h in range(HPP):
                            pt = ps.tile([128, 512], F32, tag="ps", name="psB2")
                            for pj in range(6):
                                nc.tensor.matmul(
                                    pt[:],
                                    wqu4[:, pj, :, h * 128:(h + 1) * 128],
                                    qcp8[pj][:, :, sc * 512:(sc + 1) * 512],
                                    start=(pj == 0), stop=(pj == 5),
                                    perf_mode=DR)
                            nc.scalar.activation(
                                qpair[h][:, 0, sc * 512:(sc + 1) * 512], pt[:],
                                AF.Identity,
                                bias=bias_ap("b_qu", HPP * p + h))

                # Pre-issue next-pass weight DMAs (overlap Phase C) and,
                # before C(1), prefetch Phase D's first tiles (ctx_d pass-0
                # data is complete by end of C(0)).
                if p == 0:
                    passW[1] = issue_pass_weights(1)
                else:
                    nc.gpsimd.dma_start(wo_t0[:], slab("w_o", 0))
                    ctx_r_pre = ctx_d[:].rearrange("(h hp) s -> hp h s", hp=128)
                    for st in range(2):
                        nc.gpsimd.dma_start(
                            ctx_pre[st][:],
                            ctx_r_pre[:, :, st * 128:(st + 1) * 128])

                # ---- Phase C: causal attention, transposed-scores formulation.
                # scoresT[k, q] via ONE fp8 DoubleRow matmul per block (c-part
                # and rope-part are the two halves); PT = exp(scale * .) bf16;
                # ctxT[d, q] += V_j^T PT_j (bf16); den[1, q] += ones^T PT_j;
                # ctxT normalized by 1/den on eviction, parked in ctx_d (pass
                # 0) or SBUF (pass 1).
                with ExitStack() as s:
                    PT_p = s.enter_context(tc.tile_pool(name="PTp", bufs=4))
                    sm = s.enter_context(tc.tile_pool(name="smC", bufs=4))
                    ps_sc = s.enter_context(tc.tile_pool(name="ps_sc", bufs=3, space="PSUM"))
                    ps_cx = s.enter_context(tc.tile_pool(name="ps_cx", bufs=2, space="PSUM"))
                    ps_dn = s.enter_context(tc.tile_pool(name="ps_dn", bufs=2, space="PSUM"))
                    ps_bc = s.enter_context(tc.tile_pool(name="ps_bc", bufs=1, space="PSUM"))
                    if p == 1:
                        for h in range(HPP):
                            ctx1[h] = ctx1_pool.tile(
                                [128, S], BF16, tag=f"ctx1_{h}", name=f"ctx1_{h}")
                    fin_prev = None
                    for g in range(4):
                        for h in range(HPP):
                            qlo = g * 512
                            pcx = ps_cx.tile([128, 512], F32, tag="ctx", name="pcx")
                            pden = ps_dn.tile([1, 512], F32, tag="den", name="pden")
                            njs = 4 * g + 4
                            # software-pipelined by one j: the PV/den matmuls
                            # for block j issue after block j+1's score
                            # matmuls, hiding the Exp latency from the PE.
                            pend = None

                            def flush(last):
                                jj, PTp_, c0p = pend
                                nc.tensor.matmul(
                                    pcx[:, c0p:512],
                                    V_all[:, jj * 512 + h * 128:
                                          jj * 512 + (h + 1) * 128],
                                    PTp_[:, c0p:512],
                                    start=(jj == 0), stop=last)
                                nc.tensor.matmul(
                                    pden[:, c0p:512], onesc[:], PTp_[:, c0p:512],
                                    start=(jj == 0), stop=last)

                            for j in range(njs):
                                c0 = max(0, j - 4 * g) * 128
                                pS = ps_sc.tile([128, 512], F32, tag="sT", name="pS")
                                nc.tensor.matmul(
                                    pS[:, c0:512],
                                    kpair[h][:, :, j * 128:(j + 1) * 128],
                                    qpair[h][:, :, qlo + c0:qlo + 512],
                                    start=True, stop=True, perf_mode=DR)
                                if j >= 4 * g:   # diagonal block
                                    nc.vector.tensor_add(
                                        pS[:, c0:c0 + 128], pS[:, c0:c0 + 128],
                                        causal_t)
                                PTt = PT_p.tile([128, 512], BF16, tag="PT", name="PTt")
                                nc.scalar.activation(
                                    PTt[:, c0:512], pS[:, c0:512], AF.Exp,
                                    scale=SCALE)
                                if pend is not None:
                                    flush(False)
                                pend = (j, PTt, c0)
                            flush(True)
                            rden = sm.tile([1, 512], BF16, tag="rden", name="rden")
                            with nc.allow_low_precision(
                                    reason="softmax rdenom as bf16 matmul operand"):
                                nc.vector.reciprocal(rden[:], pden[:])
                            # Finalization (rden broadcast + normalize) of the
                            # PREVIOUS group runs now: the pbc matmul never
                            # stalls the PE waiting on the DVE reciprocal.
                            if fin_prev is not None:
                                fin_prev()

                            def make_fin(pcx, rden, h, qlo):
                                def fin():
                                    pbc = ps_bc.tile([128, 512], F32, tag="bc",
                                                     name="pbc")
                                    nc.tensor.matmul(pbc[:], ones[:], rden[:],
                                                     start=True, stop=True)
                                    # DVE copy, NOT scalar.copy: the
                                    # Activation engine must stay on the Exp
                                    # table through all of C (each
                                    # Exp<->Identity switch costs ~1.3us).
                                    denb = sm.tile([128, 512], F32, tag="denb",
                                                   name="denb")
                                    nc.vector.tensor_copy(denb[:], pbc[:])
                                    if p == 1:
                                        nc.vector.tensor_mul(
                                            ctx1[h][:, qlo:qlo + 512],
                                            pcx[:], denb[:])
                                    else:
                                        cev = sm.tile([128, 512], BF16,
                                                      tag="cev", name="cev")
                                        nc.vector.tensor_mul(
                                            cev[:], pcx[:], denb[:])
                                        nc.sync.dma_start(
                                            ctx_d[h * 128:(h + 1) * 128,
                                                  qlo:qlo + 512], cev[:])
                                return fin

                            fin_prev = make_fin(pcx, rden, h, qlo)
                    fin_prev()

            # ---- Phase D: output projection (row-parallel partial, 8 heads).
            with ExitStack() as s:
                wop = s.enter_context(tc.tile_pool(name="wo", bufs=1))
                cxp = s.enter_context(tc.tile_pool(name="cxD", bufs=3))
                evd = s.enter_context(tc.tile_pool(name="evD", bufs=4))
                ps = s.enter_context(tc.tile_pool(name="psD", bufs=2, space="PSUM"))
                wo_t = [wo_t0] + [wop.tile([128, HID], BF16, tag=f"wo{h}",
                                           name=f"wo{h}")
                                  for h in range(1, HPC)]
                ctx_r = ctx_d[:].rearrange("(h hp) s -> hp h s", hp=128)
                ctx_tiles = ctx_pre
                for h in range(1, HPC):
                    nc.sync.dma_start(wo_t[h][:], slab("w_o", h))
                for st in range(NB):
                    if st < 2:
                        ctx_st = ctx_tiles[st]
                    else:
                        ctx_st = cxp.tile([128, HPP, 128], BF16, tag="cx",
                                          name="ctx_st")
                        nc.sync.dma_start(
                            ctx_st[:], ctx_r[:, :, st * 128:(st + 1) * 128])
                    pts = [ps.tile([128, 512], F32, tag=f"ps{oc}",
                                   name=f"psD{oc}") for oc in range(4)]
                    for h in range(HPC):
                        stat = (ctx_st[:, h, :] if h < HPP else
                                ctx1[h - HPP][:, st * 128:(st + 1) * 128])
                        for oc in range(4):
                            nc.tensor.matmul(
                                pts[oc][:], stat,
                                wo_t[h][:, oc * 512:(oc + 1) * 512],
                                start=(h == 0), stop=(h == HPC - 1))
                    for oc in range(4):
                        ev = evd.tile([128, 512], BF16, tag="evD", name="evD")
                        nc.scalar.copy(ev[:], pts[oc][:])
                        nc.sync.dma_start(
                            out_p.ap()[st * 128:(st + 1) * 128,
                                       oc * 512:(oc + 1) * 512], ev[:])

    nc.compile()
    return nc


def _host_inputs(inputs):
    import ml_dtypes
    f32 = np.float32
    bf16 = ml_dtypes.bfloat16
    fp8 = ml_dtypes.float8_e4m3

    def b16(a):
        return np.ascontiguousarray(np.asarray(a, f32).astype(bf16))

    def b8(a):
        return np.ascontiguousarray(np.asarray(a, f32).astype(fp8))

    x = np.asarray(inputs["x"], dtype=f32)
    W_kvd, b_kvd = inputs["W_kvd"], np.asarray(inputs["b_kvd"], f32)
    W_ku, b_ku = inputs["W_ku"], np.asarray(inputs["b_ku"], f32)
    W_vu, b_vu = inputs["W_vu"], np.asarray(inputs["b_vu"], f32)
    W_kr, b_kr = inputs["W_kr"], np.asarray(inputs["b_kr"], f32)
    W_qd, b_qd = inputs["W_qd"], np.asarray(inputs["b_qd"], f32)
    W_qu, b_qu = inputs["W_qu"], np.asarray(inputs["b_qu"], f32)
    W_qr, b_qr = inputs["W_qr"], np.asarray(inputs["b_qr"], f32)
    W_o = inputs["W_o"]

    xT = [np.ascontiguousarray(np.asarray(x[b]).T) for b in range(B)]

    inv_freq = (1.0 / (10000.0 ** (np.arange(0, RD, 2, dtype=np.float64) / RD)))
    ang = np.arange(S, dtype=np.float64)[:, None] * inv_freq[None, :]  # [S, 32]
    cosT = np.cos(ang).T.astype(f32)   # [32, S]
    sinT = np.sin(ang).T.astype(f32)
    cospair = b16(np.tile(cosT, (4, 1)))                               # [128, S]
    sinpair = b16(np.concatenate([-sinT, sinT, -sinT, sinT], axis=0))  # [128, S]
    # transposed-scores causal mask: mask k > q within the diagonal block
    causal = np.where(np.tril(np.ones((128, 128), bool), -1),
                      f32(-1e9), f32(0.0)).astype(f32)

    def tile_pack(W, n_ot):
        # [K, n_ot*cols] -> [n_ot, 128, (K/128)*cols] cc-major stationary
        W = np.asarray(W, f32)
        K, C = W.shape
        ncc = K // 128
        cols = C // n_ot
        return (W.reshape(ncc, 128, n_ot, cols).transpose(2, 1, 0, 3)
                .reshape(n_ot, 128, ncc * cols))

    def pack8(W, n_ot):
        # fp8 pair-stationary: [n_ot, 128, ncc*cols] -> [n_ot, 128, ncc/2,
        # 2, cols] flattened (cc pairs interleaved per DoubleRow half)
        t = tile_pack(W, n_ot)
        n, p, cc = t.shape
        return b8(t).reshape(n, p, -1)   # layout already cc-major pairs

    kvdT = b16(tile_pack(W_kvd, 4))       # [4, 128, 2048]
    qdT8 = pack8(W_qd, 12)                # [12, 128, 2048] (8 pairs x 2 x 128)
    b_down_c = b_kvd.reshape(4, 128).T    # [128, 4] (kv slabs only)
    b_qd_c = b_qd.reshape(12, 128).T      # [128, 12]

    in_maps = []
    for c in range(NCORES):
        b, g = c // 2, c % 2
        hc = slice(HPC * g * HD, (HPC * g + HPC) * HD)    # head cols (128 each)
        rc = slice(HPC * g * RD, (HPC * g + HPC) * RD)    # rope cols (64 each)
        consts_f = np.concatenate([
            np.pad(b_down_c, ((0, 0), (0, 12))),          # b_down: 16 cols
            np.ascontiguousarray(b_ku[hc].reshape(HPC, 128).T),
            np.ascontiguousarray(b_kr[rc].reshape(HPC // 2, 128).T),
            np.ascontiguousarray(b_qu[hc].reshape(HPC, 128).T),
            np.ascontiguousarray(b_qr[rc].reshape(HPC // 2, 128).T),
            b_qd_c,                                        # 12 cols
            causal,
        ], axis=1).astype(f32)                             # [128, 184]
        assert consts_f.shape == (128, _NCONSTF), consts_f.shape
        parts = {
            "xT": b16(xT[b]),
            "w_down": kvdT,
            "w_vu": b16(tile_pack(np.asarray(W_vu, f32)[:, hc], NPASS)),
            "w_o": b16(np.asarray(W_o, f32)[hc, :]),
            "cos": cospair,
            "sin": sinpair,
            "consts": np.ascontiguousarray(consts_f).view(bf16),
        }
        perm = np.zeros((128, 128), np.float32)
        for pp in range(128):
            perm[pp ^ 32, pp] = 1.0
        parts8 = {
            "perm8": b8(perm),
            "wqd8": qdT8,
            "wku8": pack8(np.asarray(W_ku, f32)[:, hc], NPASS),
            "wkr8": pack8(np.asarray(W_kr, f32)[:, rc], NPASS),
            "wqu8": pack8(np.asarray(W_qu, f32)[:, hc], NPASS),
            "wqr8": pack8(np.asarray(W_qr, f32)[:, rc], NPASS),
        }
        import ml_dtypes as _md
        blob = np.empty(NBLOB, bf16)
        off = 0
        for nm, r, cc_, n in _LAYOUT:
            a = np.ascontiguousarray(parts[nm]).reshape(-1)
            assert a.size == r * cc_ * n, (nm, a.size, r * cc_ * n)
            blob[off:off + a.size] = a
            off += a.size
        blob8 = np.empty(NBLOB8, fp8)
        off = 0
        for nm, r, cc_, n in _LAYOUT8:
            a = np.ascontiguousarray(parts8[nm]).reshape(-1)
            assert a.size == r * cc_ * n, (nm, a.size, r * cc_ * n)
            blob8[off:off + a.size] = a
            off += a.size
        in_maps.append({"blob": blob, "blob8": blob8})
    # V-bias fold: sum_k P = 1 after softmax, so +b_vu on V adds exactly
    # b_vu @ W_o to every output row; host adds it with b_o.
    b_eff = (np.asarray(inputs["b_o"], f32)
             + b_vu.astype(f32) @ np.asarray(W_o, f32))
    return in_maps, b_eff


def _run(inputs, trace=False):
    from concourse import bass_utils
    if "nc" not in _CACHE:
        _CACHE["nc"] = _build_nc()
    nc = _CACHE["nc"]
    in_maps, b_eff = _host_inputs(inputs)
    res = bass_utils.run_bass_kernel_spmd(
        nc, in_maps, core_ids=list(range(NCORES)), trace=trace)
    out = np.zeros((B, S, HID), np.float32)
    for c in range(NCORES):
        out[c // 2] += np.asarray(res.results[c]["out_p"], np.float32)
    out += b_eff[None, None, :]
    return out, res


def kernel(**inputs) -> np.ndarray:
    out, _ = _run(inputs, trace=False)
    return out


def bench(inputs, iters=10):
    """Time NEFF execution on the cores via PJRT, excluding host->device
    transfers and compile. Returns (best_ns, info)."""
    import time
    import jax
    from jax.experimental.shard_map import shard_map
    from jax.sharding import Mesh, PartitionSpec
    import concourse.mybir as mybir
    from concourse.bass2jax import (_bass_exec_p, install_neuronx_cc_hook,
                                    partition_id_tensor)

    if "nc" not in _CACHE:
        _CACHE["nc"] = _build_nc()
    nc = _CACHE["nc"]
    in_maps, _ = _host_inputs(inputs)
    install_neuronx_cc_hook()

    partition_name = nc.partition_id_tensor.name if nc.partition_id_tensor else None
    in_names, out_names, out_avals, zero_outs = [], [], [], []
    for alloc in nc.m.functions[0].allocations:
        if not isinstance(alloc, mybir.MemoryLocationSet):
            continue
        name = alloc.memorylocations[0].name
        if alloc.kind == "ExternalInput":
            if name != partition_name:
                in_names.append(name)
        elif alloc.kind == "ExternalOutput":
            out_names.append(name)
            shape = tuple(alloc.tensor_shape)
            dtype = mybir.dt.np(alloc.dtype)
            out_avals.append(jax.core.ShapedArray(shape, dtype))
            zero_outs.append(np.zeros(shape, dtype))
    n_params = len(in_names)
    all_names = list(in_names) + list(out_names)
    if partition_name is not None:
        all_names.append(partition_name)

    def _body(*args):
        operands = list(args)
        if partition_name is not None:
            operands.append(partition_id_tensor())
        outs = _bass_exec_p.bind(
            *operands,
            out_avals=tuple(out_avals),
            in_names=tuple(all_names),
            out_names=tuple(out_names),
            lowering_input_output_aliases=(),
            sim_require_finite=True,
            sim_require_nnan=True,
            nc=nc,
        )
        return tuple(outs)

    n = NCORES
    devices = jax.devices()[:n]
    mesh = Mesh(np.asarray(devices), ("core",))
    nin = n_params + len(out_names)
    fn = jax.jit(shard_map(
        _body, mesh=mesh,
        in_specs=(PartitionSpec("core"),) * nin,
        out_specs=(PartitionSpec("core"),) * len(out_names),
        check_rep=False), keep_unused=True)
    concat_in = [np.concatenate([np.asarray(in_maps[c][k]) for c in range(n)], 0)
                 for k in in_names]
    concat_zeros = [np.zeros((n * z.shape[0], *z.shape[1:]), z.dtype)
                    for z in zero_outs]
    sharding = jax.sharding.NamedSharding(mesh, PartitionSpec("core"))
    dev_in = [jax.device_put(a, sharding) for a in concat_in + concat_zeros]
    out = fn(*dev_in)  # warm-up/compile
    jax.block_until_ready(out)
    times = []
    for _ in range(iters):
        t0 = time.perf_counter()
        out = fn(*dev_in)
        jax.block_until_ready(out)
        times.append((time.perf_counter() - t0) * 1e9)

    def run_k(k):
        t0 = time.perf_counter()
        outs = [fn(*dev_in) for _ in range(k)]
        jax.block_until_ready(outs)
        return (time.perf_counter() - t0) * 1e9

    # pipelined: K async submissions, block once; amortizes tunnel latency.
    K1, K2 = 8, 48
    piped_samples, tKs = [], []
    for _ in range(7):
        a = run_k(K1)
        b = run_k(K2)
        tKs.append((a, b))
        piped_samples.append((b - a) / (K2 - K1))
    piped = sorted(piped_samples)[len(piped_samples) // 2]
    sustained = min(b / K2 for _, b in tKs)
    t1 = min(times)
    best = min(times + [sustained])
    if 0 < piped < sustained:
        best = min(best, piped)
    return best, {"serial": times, "tK": tKs[-1][1], "t1": t1,
                  "piped": piped, "piped_samples": piped_samples,
                  "sustained": sustained}
